# revision 15
# baseline (speedup 1.0000x reference)
"""Trainium2 Bass kernel for moe_routing (nn_CITADEL_15118284882566).

Math: the reference collapses (qw >= 0; the max over Ld*Kd always
includes exact-zero entries from non-matches) to, per pair b:

    out[b] = sum_q qw[b,q] * relu( max_{l,kd} sims[b,q,l] * dw[b,l,kd]
                                   * [d_id[b,l,kd] == q_id[b,q]] )
             + dot(q_cls[b], d_cls[b])

Device strategy (data-parallel over B across 8 cores, 64 pairs/core,
16 groups of 4 pairs; partitions = 4 pairs x 32 queries):

1. NEGSQ: v = eps*dw - sum_i (dcomp_i - qcomp_i)^2 via K-stacked fp16
   matmuls (columns l-major: col = l*5+kd).  Ids split into four 4-bit
   components so every operand is fp16-exact and all partial sums are
   small integers (exact in f32); dw rows accumulate last so matches
   give v == eps*fp16(dw) EXACTLY; non-matches give v <= -0.999.
   The 25-row matmuls are row-tiled 4-way (tile_position=(32a,0)) so
   they run concurrently in the PE array.
2. sims via 4 column-tiled matmuls (contraction over D=128), also
   concurrent.
3. Drain: relu(2^20 * v) -> fp16: matches 2^8*dw, non-matches 0.
   Chunks 0-3 (psA) on ACT; chunk 4 (psB) on DVE ts (mult,max).
4. kd-max: ONE pool_max over [p][l][5] (kd contiguous innermost).
5. ONE tensor_tensor_reduce: prd = s_ps * dmx, res_raw[:,g] =
   max(prd, init=0) -- fuses multiply, l-reduce and the relu clamp.
6. Epilogue: res = res_raw*qw, tok sums via one-hot matmul (2^-8),
   cls dots via elementwise mult + ones matmul; host adds the outputs.
"""
import sys

sys.path.insert(0, "/opt/trn_rl_repo")

import numpy as np

B, LQ, LD, KQ, KD, D = 512, 32, 512, 1, 5, 128
NCORES = 8
BPC = B // NCORES          # 64 pairs per core
NB = 4                     # pairs per group
G = BPC // NB              # 16 groups
P = 128
JD = KD * LD               # 2560
KS = 25                    # 4 d-quad + 16 d-lin + 1 q-quad + 4 dw
EPS = 2.0 ** -12
SCALE = 2.0 ** 20
JA = 4 * LD                # cols drained by ACT (kd 0-3)
JB = JD - JA               # kd-4 chunk, consumed straight from PSUM
RW = 768                   # rhx width: 512 rhs + 128 chunk4 + 128 lhsT

_CACHED = {}
FLAT_RHX = True

DEFAULT_OPTS = dict(
    io_bufs=5,
    big_bufs=3,
    rowtile=False,      # 4-way row-tiled concurrent diff matmuls
    use_ttr=True,      # fused prd+reduce (else tt + reduce)
    dT_eng="gpsimd",
    rhx_eng="gpsimd",
)


def _build_module(**kw):
    opts = dict(DEFAULT_OPTS)
    opts.update(kw)
    import concourse.bacc as bacc
    import concourse.mybir as mybir
    from concourse import tile

    f16 = mybir.dt.float16
    f32 = mybir.dt.float32
    Alu = mybir.AluOpType
    Act = mybir.ActivationFunctionType

    nc = bacc.Bacc("TRN2", target_bir_lowering=False, debug=False)

    def eng(name):
        return {"gpsimd": nc.gpsimd, "sync": nc.sync, "scalar": nc.scalar,
                "vector": nc.vector, "tensor": nc.tensor}[name]

    qdT_d = nc.dram_tensor("qdT", [G, D, NB * LQ + NB * LD], f16, kind="ExternalInput")
    flat = opts.get("flat_rhx", False)
    rhx_d = (nc.dram_tensor("rhx", [G, KS, JD + P], f16, kind="ExternalInput")
             if flat else
             nc.dram_tensor("rhx", [G, P, RW], f16, kind="ExternalInput"))
    epi_d = nc.dram_tensor("epi", [P, 2 * BPC + NB + 1 + G], f32, kind="ExternalInput")

    tok_d = nc.dram_tensor("tok", [NB, G], f32, kind="ExternalOutput")
    cls_d = nc.dram_tensor("cls", [1, BPC], f32, kind="ExternalOutput")

    with tile.TileContext(nc) as tc:
        with (
            tc.tile_pool(name="sb_io", bufs=opts["io_bufs"]) as sb_io,
            tc.tile_pool(name="sb_big", bufs=opts["big_bufs"]) as sb_big,
            tc.tile_pool(name="sb_wk", bufs=2) as sb_wk,
            tc.tile_pool(name="sb_res", bufs=1) as sb_res,
            tc.tile_pool(name="ps_a", bufs=1, space="PSUM") as ps_a,
            tc.tile_pool(name="ps_a2", bufs=1, space="PSUM") as ps_a2,
            tc.tile_pool(name="ps_b", bufs=2, space="PSUM") as ps_b,
            tc.tile_pool(name="ps_s", bufs=2, space="PSUM") as ps_s,
        ):
            res_raw = sb_res.tile([P, G], f32)
            epi_t = sb_res.tile([P, 2 * BPC + NB + 1 + G], f32)
            nc.sync.dma_start(epi_t[:], epi_d[:])
            qw_all = epi_t[:, 2 * BPC + NB + 1:]

            for gp in range(G // 2):
                psBs, s_pss = [], []
                for h in (0, 1):
                    g = 2 * gp + h
                    qdT_t = sb_io.tile([D, NB * LQ + NB * LD], f16, name="qdT_t")
                    rhx_t = (sb_io.tile([KS, JD + P], f16, name="rhx_t") if flat
                             else sb_io.tile([P, RW], f16, name="rhx_t"))
                    eng(opts["dT_eng"]).dma_start(qdT_t[:], qdT_d[g, :, :])
                    eng(opts["rhx_eng"]).dma_start(rhx_t[:], rhx_d[g, :, :])
                    qTx_t = qdT_t[:, 0:NB * LQ]
                    dT_t = qdT_t[:, NB * LQ:]

                    s_ps = ps_s.tile([P, LD], f32, name="s_ps", tag="spool")
                    for b in range(NB):
                        nc.tensor.matmul(
                            s_ps[b * LQ:(b + 1) * LQ, :],
                            qTx_t[:, b * LQ:(b + 1) * LQ],
                            dT_t[:, b * LD:(b + 1) * LD],
                            start=True, stop=True,
                            tile_position=(0, b * LQ),
                        )
                    s_pss.append(s_ps)

                    psA1 = ps_a.tile([P, 2 * LD], f32, name="psA1")
                    psA2 = ps_a2.tile([P, 2 * LD], f32, name="psA2")
                    psB = ps_b.tile([P, JB], f32, name="psB")
                    psBs.append(psB)
                    for a in range(4):
                        dst = psA1 if a < 2 else psA2
                        nc.tensor.matmul(
                            dst[:, (a % 2) * LD:(a % 2 + 1) * LD],
                            rhx_t[:, JD:JD + P],
                            rhx_t[:, a * LD:(a + 1) * LD],
                            start=True, stop=True,
                        )
                    nc.tensor.matmul(
                        psB[:], rhx_t[:, JD:JD + P], rhx_t[:, JA:JD],
                        start=True, stop=True)

                    if h == 0:
                        d2sA2 = sb_big.tile([P, 4 * LD], f16, name="d2sA2")
                        d2sB2 = sb_big.tile([P, 4 * LD], f16, name="d2sB2")
                    nc.scalar.activation(d2sA2[:, h * 2 * LD:(h + 1) * 2 * LD],
                                         psA1[:], Act.Relu, bias=0.0, scale=SCALE)
                    nc.scalar.activation(d2sB2[:, h * 2 * LD:(h + 1) * 2 * LD],
                                         psA2[:], Act.Relu, bias=0.0, scale=SCALE)

                # batched kd-tree over both groups: [p][u=2][k=2][l]
                vA = d2sA2.rearrange("p (u k l) -> p u k l", u=2, k=2)
                vB = d2sB2.rearrange("p (u k l) -> p u k l", u=2, k=2)
                t01 = sb_wk.tile([P, 2 * LD], f16, name="t01")
                t23 = sb_wk.tile([P, 2 * LD], f16, name="t23")
                v01 = t01.rearrange("p (u l) -> p u l", u=2)
                v23 = t23.rearrange("p (u l) -> p u l", u=2)
                nc.vector.tensor_tensor(v01[:], vA[:, :, 0, :], vA[:, :, 1, :], Alu.max)
                nc.vector.tensor_tensor(v23[:], vB[:, :, 0, :], vB[:, :, 1, :], Alu.max)
                nc.vector.tensor_tensor(t01[:], t01[:], t23[:], Alu.max)

                dmx = sb_wk.tile([P, 2 * LD], f16, name="dmx")
                prd = sb_wk.tile([P, 2 * LD], f16, name="prd")
                for h in (0, 1):
                    sl = slice(h * LD, (h + 1) * LD)
                    nc.vector.scalar_tensor_tensor(
                        dmx[:, sl], psBs[h][:], SCALE, t01[:, sl], Alu.mult, Alu.max)
                    nc.vector.tensor_tensor(prd[:, sl], s_pss[h][:], dmx[:, sl], Alu.mult)
                nc.vector.reduce_max(
                    res_raw[:, 2 * gp:2 * gp + 2],
                    prd.rearrange("p (u l) -> p u l", u=2)[:],
                    axis=mybir.AxisListType.X)

            # ---- epilogue: *qw, tok colsums, cls dots ----
            qcT_t = epi_t[:, 0:BPC]
            dcT_t = epi_t[:, BPC:2 * BPC]
            e4s_t = epi_t[:, 2 * BPC:2 * BPC + NB]
            ones_t = epi_t[:, 2 * BPC + NB:2 * BPC + NB + 1]

            res = sb_res.tile([P, G], f32)
            nc.vector.tensor_tensor(res[:], res_raw[:], qw_all, Alu.mult)

            cp = sb_res.tile([D, BPC], f32)
            nc.vector.tensor_tensor(cp[:], qcT_t, dcT_t, Alu.mult)

            tok_ps = ps_s.tile([NB, G], f32, name="tok_ps", tag="spool")
            nc.tensor.matmul(tok_ps[:], e4s_t, res[:], start=True, stop=True)
            cls_ps = ps_s.tile([1, BPC], f32, name="cls_ps", tag="spool")
            nc.tensor.matmul(cls_ps[:], ones_t, cp[:], start=True, stop=True)

            tok_sb = sb_res.tile([NB, G], f32)
            cls_sb = sb_res.tile([1, BPC], f32)
            nc.vector.tensor_copy(tok_sb[:], tok_ps[:])
            nc.vector.tensor_copy(cls_sb[:], cls_ps[:])
            nc.gpsimd.dma_start(tok_d[:], tok_sb[:])
            nc.gpsimd.dma_start(cls_d[:], cls_sb[:])

    nc.compile()
    return nc


def _comps(x):
    """Four 4-bit components of int ids (values 0..30521 < 2^15)."""
    return [(x >> 12) & 15, (x >> 8) & 15, (x >> 4) & 15, x & 15]


def _prep_core_inputs(c, q_repr, q_w, q_ids, q_cls, d_repr, d_w, d_ids, d_cls):
    """Pure layout/packing for one core's 64 pairs."""
    s = slice(c * BPC, (c + 1) * BPC)
    qr = q_repr[s]          # [64, 32, 128] f32
    qw = q_w[s, :, 0]       # [64, 32]
    qi = q_ids[s, :, 0]     # [64, 32] int64
    qc = q_cls[s]           # [64, 128]
    dr = d_repr[s]          # [64, 512, 128]
    dw = d_w[s]             # [64, 512, 5]
    di = d_ids[s]           # [64, 512, 5]
    dc = d_cls[s]           # [64, 128]

    qT = np.ascontiguousarray(
        qr.reshape(G, NB, LQ, D).transpose(0, 3, 1, 2).reshape(G, D, NB * LQ))
    dT = np.ascontiguousarray(
        dr.reshape(G, NB, LD, D).transpose(0, 3, 1, 2).reshape(G, D, NB * LD))
    qdT = np.concatenate([qT, dT], axis=2).astype(np.float16)
    qww = qw.reshape(G, NB * LQ)  # partition p = 32*b + q

    qcs = [a.astype(np.float32) for a in _comps(qi)]        # each [64, 32]
    dcs = [a.astype(np.float32) for a in _comps(di)]        # each [64, 512, 5]
    dsq = sum(a * a for a in dcs)                           # [64, 512, 5]
    qsq = sum(a * a for a in qcs)                           # [64, 32]
    dw16 = dw.astype(np.float16).astype(np.float32)

    E = np.zeros((NB, P), np.float32)
    for b in range(NB):
        E[b, b * LQ:(b + 1) * LQ] = 1.0

    def dcol(a):
        # [64, 512, 5] -> [G, NB, JD] kd-major (col = kd*512 + l)
        return a.reshape(G, NB, LD, KD).transpose(0, 1, 3, 2).reshape(G, NB, JD)

    # full-width rhs [G, KS, JD] + lhsT [G, KS, P], then fold into blocks
    rhs = np.zeros((G, KS, JD), np.float32)
    lhsT = np.zeros((G, KS, P), np.float32)
    rhs[:, 0:4, :] = dcol(dsq)
    lhsT[:, 0:4, :] = -E
    for i in range(4):
        rhs[:, 4 + 4 * i:8 + 4 * i, :] = dcol(dcs[i])
        lhsT[:, 4 + 4 * i:8 + 4 * i, :] = (2.0 * qcs[i].reshape(G, P))[:, None, :] * E
    rhs[:, 20, :] = 1.0
    lhsT[:, 20, :] = -qsq.reshape(G, P)
    rhs[:, 21:25, :] = dcol(dw16)
    lhsT[:, 21:25, :] = EPS * E

    if FLAT_RHX:
        rhx = np.zeros((G, KS, JD + P), np.float32)
        rhx[:, :, 0:JD] = rhs
        rhx[:, :, JD:] = lhsT
    else:
        # rhx: [G, P, RW]; block a = partitions 32a..32a+KS:
        #   cols 0:512 = rhs chunk a, 512:640 = chunk-4 sub a, 640:768 = lhsT
        rhx = np.zeros((G, P, RW), np.float32)
        for a in range(4):
            blk = slice(32 * a, 32 * a + KS)
            rhx[:, blk, 0:512] = rhs[:, :, 512 * a:512 * (a + 1)]
            rhx[:, blk, 512:640] = rhs[:, :, JA + 128 * a:JA + 128 * (a + 1)]
            rhx[:, blk, 640:768] = lhsT

    epi = np.zeros((P, 2 * BPC + NB + 1 + G), np.float32)
    epi[:, 0:BPC] = qc.T
    epi[:, BPC:2 * BPC] = dc.T
    for b in range(NB):
        epi[b * LQ:(b + 1) * LQ, 2 * BPC + b] = 2.0 ** -8
    epi[:, 2 * BPC + NB] = 1.0
    epi[:, 2 * BPC + NB + 1:] = qww.T

    return {
        "qdT": qdT,
        "rhx": rhx.astype(np.float16),
        "epi": epi,
    }


def kernel(q_expert_repr, q_expert_weights, q_expert_ids, q_cls_repr,
           d_expert_repr, d_expert_weights, d_expert_ids, d_cls_repr):
    from concourse.bass_utils import run_bass_kernel_spmd

    q_repr = np.asarray(q_expert_repr, np.float32)
    q_w = np.asarray(q_expert_weights, np.float32)
    q_ids = np.asarray(q_expert_ids, np.int64)
    q_cls = np.asarray(q_cls_repr, np.float32)
    d_repr = np.asarray(d_expert_repr, np.float32)
    d_w = np.asarray(d_expert_weights, np.float32)
    d_ids = np.asarray(d_expert_ids, np.int64)
    d_cls = np.asarray(d_cls_repr, np.float32)

    if "nc" not in _CACHED:
        _CACHED["nc"] = _build_module(flat_rhx=FLAT_RHX, use_ttr=False)
    nc = _CACHED["nc"]

    in_maps = [
        _prep_core_inputs(c, q_repr, q_w, q_ids, q_cls, d_repr, d_w, d_ids, d_cls)
        for c in range(NCORES)
    ]
    rr = run_bass_kernel_spmd(nc, in_maps, core_ids=list(range(NCORES)))

    out = np.zeros((B,), np.float32)
    for c in range(NCORES):
        tok = rr.results[c]["tok"]          # [NB, G]
        cls = rr.results[c]["cls"][0]       # [BPC]
        out[c * BPC:(c + 1) * BPC] = tok.T.reshape(-1) + cls
    return out


if __name__ == "__main__":
    rng = np.random.default_rng(0)
    ins = {
        "q_expert_repr": rng.standard_normal((B, LQ, D)).astype(np.float32),
        "q_expert_weights": rng.random((B, LQ, KQ)).astype(np.float32),
        "q_expert_ids": rng.integers(0, 30522, (B, LQ, KQ)).astype(np.int64),
        "q_cls_repr": rng.standard_normal((B, D)).astype(np.float32),
        "d_expert_repr": rng.standard_normal((B, LD, D)).astype(np.float32),
        "d_expert_weights": rng.random((B, LD, KD)).astype(np.float32),
        "d_expert_ids": rng.integers(0, 30522, (B, LD, KD)).astype(np.int64),
        "d_cls_repr": rng.standard_normal((B, D)).astype(np.float32),
    }
    out = kernel(**ins)
    print("kernel out[:8]:", out[:8])


# revision 17
# speedup vs baseline: 1.1145x; 1.1145x over previous
"""Trainium2 Bass kernel for moe_routing (nn_CITADEL_15118284882566).

Math: the reference collapses (qw >= 0; the max over Ld*Kd always
includes exact-zero entries from non-matches) to, per pair b:

    out[b] = sum_q qw[b,q] * relu( max_{l,kd} sims[b,q,l] * dw[b,l,kd]
                                   * [d_id[b,l,kd] == q_id[b,q]] )
             + dot(q_cls[b], d_cls[b])

Device strategy (data-parallel over B across 8 cores, 64 pairs/core,
16 groups of 4 pairs; partitions = 4 pairs x 32 queries):

1. NEGSQ: v = eps*dw - sum_i (dcomp_i - qcomp_i)^2 via K-stacked fp16
   matmuls (columns l-major: col = l*5+kd).  Ids split into four 4-bit
   components so every operand is fp16-exact and all partial sums are
   small integers (exact in f32); dw rows accumulate last so matches
   give v == eps*fp16(dw) EXACTLY; non-matches give v <= -0.999.
   The 25-row matmuls are row-tiled 4-way (tile_position=(32a,0)) so
   they run concurrently in the PE array.
2. sims via 4 column-tiled matmuls (contraction over D=128), also
   concurrent.
3. Drain: relu(2^20 * v) -> fp16: matches 2^8*dw, non-matches 0.
   Chunks 0-3 (psA) on ACT; chunk 4 (psB) on DVE ts (mult,max).
4. kd-max: ONE pool_max over [p][l][5] (kd contiguous innermost).
5. ONE tensor_tensor_reduce: prd = s_ps * dmx, res_raw[:,g] =
   max(prd, init=0) -- fuses multiply, l-reduce and the relu clamp.
6. Epilogue: res = res_raw*qw, tok sums via one-hot matmul (2^-8),
   cls dots via elementwise mult + ones matmul; host adds the outputs.
"""
import sys

sys.path.insert(0, "/opt/trn_rl_repo")

import numpy as np

B, LQ, LD, KQ, KD, D = 512, 32, 512, 1, 5, 128
NCORES = 8
BPC = B // NCORES          # 64 pairs per core
NB = 4                     # pairs per group
G = BPC // NB              # 16 groups
P = 128
JD = KD * LD               # 2560
KS = 25                    # 4 d-quad + 16 d-lin + 1 q-quad + 4 dw
EPS = 2.0 ** -12
SCALE = 2.0 ** 20
JA = 4 * LD                # cols drained by ACT (kd 0-3)
JB = JD - JA               # kd-4 chunk, consumed straight from PSUM
RW = 768                   # rhx width: 512 rhs + 128 chunk4 + 128 lhsT

_CACHED = {}
FLAT_RHX = True

DEFAULT_OPTS = dict(
    io_bufs=5,
    big_bufs=3,
    rowtile=False,      # 4-way row-tiled concurrent diff matmuls
    use_ttr=True,      # fused prd+reduce (else tt + reduce)
    dT_eng="gpsimd",
    rhx_eng="gpsimd",
)


def _build_module(**kw):
    opts = dict(DEFAULT_OPTS)
    opts.update(kw)
    import concourse.bacc as bacc
    import concourse.mybir as mybir
    from concourse import tile

    f16 = mybir.dt.float16
    f32 = mybir.dt.float32
    Alu = mybir.AluOpType
    Act = mybir.ActivationFunctionType

    nc = bacc.Bacc("TRN2", target_bir_lowering=False, debug=False)

    def eng(name):
        return {"gpsimd": nc.gpsimd, "sync": nc.sync, "scalar": nc.scalar,
                "vector": nc.vector, "tensor": nc.tensor}[name]

    qdT_d = nc.dram_tensor("qdT", [G, D, NB * LQ + NB * LD], f16, kind="ExternalInput")
    flat = opts.get("flat_rhx", False)
    rhx_d = (nc.dram_tensor("rhx", [G, KS, JD + P], f16, kind="ExternalInput")
             if flat else
             nc.dram_tensor("rhx", [G, P, RW], f16, kind="ExternalInput"))
    epi_d = nc.dram_tensor("epi", [P, 2 * BPC + NB + 1 + G], f32, kind="ExternalInput")

    tok_d = nc.dram_tensor("tok", [NB, G], f32, kind="ExternalOutput")
    cls_d = nc.dram_tensor("cls", [1, BPC], f32, kind="ExternalOutput")

    with tile.TileContext(nc) as tc:
        with (
            tc.tile_pool(name="sb_io", bufs=opts["io_bufs"]) as sb_io,
            tc.tile_pool(name="sb_big", bufs=opts["big_bufs"]) as sb_big,
            tc.tile_pool(name="sb_wk", bufs=2) as sb_wk,
            tc.tile_pool(name="sb_res", bufs=1) as sb_res,
            tc.tile_pool(name="ps_a", bufs=1, space="PSUM") as ps_a,
            tc.tile_pool(name="ps_a2", bufs=1, space="PSUM") as ps_a2,
            tc.tile_pool(name="ps_b", bufs=2, space="PSUM") as ps_b,
            tc.tile_pool(name="ps_s", bufs=2, space="PSUM") as ps_s,
        ):
            res_raw = sb_res.tile([P, G], f32)
            epi_t = sb_res.tile([P, 2 * BPC + NB + 1 + G], f32)
            nc.sync.dma_start(epi_t[:], epi_d[:])
            qw_all = epi_t[:, 2 * BPC + NB + 1:]

            for g in range(G):
                qdT_t = sb_io.tile([D, NB * LQ + NB * LD], f16, name="qdT_t")
                rhx_t = (sb_io.tile([KS, JD + P], f16, name="rhx_t") if flat
                         else sb_io.tile([P, RW], f16, name="rhx_t"))
                eng(opts["dT_eng"]).dma_start(qdT_t[:], qdT_d[g, :, :])
                eng(opts["rhx_eng"]).dma_start(rhx_t[:], rhx_d[g, :, :])
                qTx_t = qdT_t[:, 0:NB * LQ]
                dT_t = qdT_t[:, NB * LQ:]

                s_ps = ps_s.tile([P, LD], f32, name="s_ps", tag="spool")
                for b in range(NB):
                    nc.tensor.matmul(
                        s_ps[b * LQ:(b + 1) * LQ, :],
                        qTx_t[:, b * LQ:(b + 1) * LQ],
                        dT_t[:, b * LD:(b + 1) * LD],
                        start=True, stop=True,
                        tile_position=(0, b * LQ),
                    )

                psA1 = ps_a.tile([P, 2 * LD], f32, name="psA1")
                psA2 = ps_a2.tile([P, 2 * LD], f32, name="psA2")
                psB = ps_b.tile([P, JB], f32, name="psB")
                for a in range(4):
                    dst = psA1 if a < 2 else psA2
                    nc.tensor.matmul(
                        dst[:, (a % 2) * LD:(a % 2 + 1) * LD],
                        rhx_t[:, JD:JD + P],
                        rhx_t[:, a * LD:(a + 1) * LD],
                        start=True, stop=True,
                    )
                nc.tensor.matmul(
                    psB[:], rhx_t[:, JD:JD + P], rhx_t[:, JA:JD],
                    start=True, stop=True)

                d2sA = sb_big.tile([P, 2 * LD], f16, name="d2sA")
                d2sB = sb_big.tile([P, 2 * LD], f16, name="d2sB")
                nc.scalar.activation(d2sA[:], psA1[:], Act.Relu,
                                     bias=0.0, scale=SCALE)
                nc.scalar.activation(d2sB[:], psA2[:], Act.Relu,
                                     bias=0.0, scale=SCALE)

                t01 = sb_wk.tile([P, LD], f16, name="t01")
                t23 = sb_wk.tile([P, LD], f16, name="t23")
                nc.vector.tensor_tensor(t01[:], d2sA[:, 0:LD], d2sA[:, LD:2 * LD], Alu.max)
                nc.vector.tensor_tensor(t23[:], d2sB[:, 0:LD], d2sB[:, LD:2 * LD], Alu.max)
                nc.vector.tensor_tensor(t01[:], t01[:], t23[:], Alu.max)
                dmx = sb_wk.tile([P, LD], f16, name="dmx")
                nc.vector.scalar_tensor_tensor(
                    dmx[:], psB[:], SCALE, t01[:], Alu.mult, Alu.max)

                prd = sb_wk.tile([P, LD], f16, name="prd")
                if opts["use_ttr"]:
                    mx = sb_wk.tile([P, 1], f32, name="mx")
                    nc.vector.tensor_tensor_reduce(
                        prd[:], s_ps[:], dmx[:], 1.0, 0.0,
                        Alu.mult, Alu.max, mx[:])
                    nc.vector.tensor_copy(res_raw[:, g:g + 1], mx[:])
                else:
                    nc.vector.tensor_tensor(prd[:], s_ps[:], dmx[:], Alu.mult)
                    nc.vector.reduce_max(res_raw[:, g:g + 1], prd[:],
                                         axis=mybir.AxisListType.X)

            # ---- epilogue: *qw, tok colsums, cls dots ----
            qcT_t = epi_t[:, 0:BPC]
            dcT_t = epi_t[:, BPC:2 * BPC]
            e4s_t = epi_t[:, 2 * BPC:2 * BPC + NB]
            ones_t = epi_t[:, 2 * BPC + NB:2 * BPC + NB + 1]

            res = sb_res.tile([P, G], f32)
            nc.vector.tensor_tensor(res[:], res_raw[:], qw_all, Alu.mult)

            cp = sb_res.tile([D, BPC], f32)
            nc.vector.tensor_tensor(cp[:], qcT_t, dcT_t, Alu.mult)

            tok_ps = ps_s.tile([NB, G], f32, name="tok_ps", tag="spool")
            nc.tensor.matmul(tok_ps[:], e4s_t, res[:], start=True, stop=True)
            cls_ps = ps_s.tile([1, BPC], f32, name="cls_ps", tag="spool")
            nc.tensor.matmul(cls_ps[:], ones_t, cp[:], start=True, stop=True)

            tok_sb = sb_res.tile([NB, G], f32)
            cls_sb = sb_res.tile([1, BPC], f32)
            nc.vector.tensor_copy(tok_sb[:], tok_ps[:])
            nc.vector.tensor_copy(cls_sb[:], cls_ps[:])
            nc.gpsimd.dma_start(tok_d[:], tok_sb[:])
            nc.gpsimd.dma_start(cls_d[:], cls_sb[:])

    nc.compile()
    return nc


def _comps(x):
    """Four 4-bit components of int ids (values 0..30521 < 2^15)."""
    return [(x >> 12) & 15, (x >> 8) & 15, (x >> 4) & 15, x & 15]


def _prep_core_inputs(c, q_repr, q_w, q_ids, q_cls, d_repr, d_w, d_ids, d_cls):
    """Pure layout/packing for one core's 64 pairs."""
    s = slice(c * BPC, (c + 1) * BPC)
    qr = q_repr[s]          # [64, 32, 128] f32
    qw = q_w[s, :, 0]       # [64, 32]
    qi = q_ids[s, :, 0]     # [64, 32] int64
    qc = q_cls[s]           # [64, 128]
    dr = d_repr[s]          # [64, 512, 128]
    dw = d_w[s]             # [64, 512, 5]
    di = d_ids[s]           # [64, 512, 5]
    dc = d_cls[s]           # [64, 128]

    qT = np.ascontiguousarray(
        qr.reshape(G, NB, LQ, D).transpose(0, 3, 1, 2).reshape(G, D, NB * LQ))
    dT = np.ascontiguousarray(
        dr.reshape(G, NB, LD, D).transpose(0, 3, 1, 2).reshape(G, D, NB * LD))
    qdT = np.concatenate([qT, dT], axis=2).astype(np.float16)
    qww = qw.reshape(G, NB * LQ)  # partition p = 32*b + q

    qcs = [a.astype(np.float32) for a in _comps(qi)]        # each [64, 32]
    dcs = [a.astype(np.float32) for a in _comps(di)]        # each [64, 512, 5]
    dsq = sum(a * a for a in dcs)                           # [64, 512, 5]
    qsq = sum(a * a for a in qcs)                           # [64, 32]
    dw16 = dw.astype(np.float16).astype(np.float32)

    E = np.zeros((NB, P), np.float32)
    for b in range(NB):
        E[b, b * LQ:(b + 1) * LQ] = 1.0

    def dcol(a):
        # [64, 512, 5] -> [G, NB, JD] kd-major (col = kd*512 + l)
        return a.reshape(G, NB, LD, KD).transpose(0, 1, 3, 2).reshape(G, NB, JD)

    # full-width rhs [G, KS, JD] + lhsT [G, KS, P], then fold into blocks
    rhs = np.zeros((G, KS, JD), np.float32)
    lhsT = np.zeros((G, KS, P), np.float32)
    rhs[:, 0:4, :] = dcol(dsq)
    lhsT[:, 0:4, :] = -E
    for i in range(4):
        rhs[:, 4 + 4 * i:8 + 4 * i, :] = dcol(dcs[i])
        lhsT[:, 4 + 4 * i:8 + 4 * i, :] = (2.0 * qcs[i].reshape(G, P))[:, None, :] * E
    rhs[:, 20, :] = 1.0
    lhsT[:, 20, :] = -qsq.reshape(G, P)
    rhs[:, 21:25, :] = dcol(dw16)
    lhsT[:, 21:25, :] = EPS * E

    if FLAT_RHX:
        rhx = np.zeros((G, KS, JD + P), np.float32)
        rhx[:, :, 0:JD] = rhs
        rhx[:, :, JD:] = lhsT
    else:
        # rhx: [G, P, RW]; block a = partitions 32a..32a+KS:
        #   cols 0:512 = rhs chunk a, 512:640 = chunk-4 sub a, 640:768 = lhsT
        rhx = np.zeros((G, P, RW), np.float32)
        for a in range(4):
            blk = slice(32 * a, 32 * a + KS)
            rhx[:, blk, 0:512] = rhs[:, :, 512 * a:512 * (a + 1)]
            rhx[:, blk, 512:640] = rhs[:, :, JA + 128 * a:JA + 128 * (a + 1)]
            rhx[:, blk, 640:768] = lhsT

    epi = np.zeros((P, 2 * BPC + NB + 1 + G), np.float32)
    epi[:, 0:BPC] = qc.T
    epi[:, BPC:2 * BPC] = dc.T
    for b in range(NB):
        epi[b * LQ:(b + 1) * LQ, 2 * BPC + b] = 2.0 ** -8
    epi[:, 2 * BPC + NB] = 1.0
    epi[:, 2 * BPC + NB + 1:] = qww.T

    return {
        "qdT": qdT,
        "rhx": rhx.astype(np.float16),
        "epi": epi,
    }


def kernel(q_expert_repr, q_expert_weights, q_expert_ids, q_cls_repr,
           d_expert_repr, d_expert_weights, d_expert_ids, d_cls_repr):
    from concourse.bass_utils import run_bass_kernel_spmd

    q_repr = np.asarray(q_expert_repr, np.float32)
    q_w = np.asarray(q_expert_weights, np.float32)
    q_ids = np.asarray(q_expert_ids, np.int64)
    q_cls = np.asarray(q_cls_repr, np.float32)
    d_repr = np.asarray(d_expert_repr, np.float32)
    d_w = np.asarray(d_expert_weights, np.float32)
    d_ids = np.asarray(d_expert_ids, np.int64)
    d_cls = np.asarray(d_cls_repr, np.float32)

    if "nc" not in _CACHED:
        _CACHED["nc"] = _build_module(flat_rhx=FLAT_RHX, use_ttr=False)
    nc = _CACHED["nc"]

    in_maps = [
        _prep_core_inputs(c, q_repr, q_w, q_ids, q_cls, d_repr, d_w, d_ids, d_cls)
        for c in range(NCORES)
    ]
    rr = run_bass_kernel_spmd(nc, in_maps, core_ids=list(range(NCORES)))

    out = np.zeros((B,), np.float32)
    for c in range(NCORES):
        tok = rr.results[c]["tok"]          # [NB, G]
        cls = rr.results[c]["cls"][0]       # [BPC]
        out[c * BPC:(c + 1) * BPC] = tok.T.reshape(-1) + cls
    return out


if __name__ == "__main__":
    rng = np.random.default_rng(0)
    ins = {
        "q_expert_repr": rng.standard_normal((B, LQ, D)).astype(np.float32),
        "q_expert_weights": rng.random((B, LQ, KQ)).astype(np.float32),
        "q_expert_ids": rng.integers(0, 30522, (B, LQ, KQ)).astype(np.int64),
        "q_cls_repr": rng.standard_normal((B, D)).astype(np.float32),
        "d_expert_repr": rng.standard_normal((B, LD, D)).astype(np.float32),
        "d_expert_weights": rng.random((B, LD, KD)).astype(np.float32),
        "d_expert_ids": rng.integers(0, 30522, (B, LD, KD)).astype(np.int64),
        "d_cls_repr": rng.standard_normal((B, D)).astype(np.float32),
    }
    out = kernel(**ins)
    print("kernel out[:8]:", out[:8])
